# revision 8
# baseline (speedup 1.0000x reference)
"""Multi-head causal self-attention on 8 TRN2 NeuronCores.

Problem: B=2, T=4096, D=512, H=8 heads (hd=64), fp32 in/out.

Sharding: core c in 0..7 handles batch b = c//4 and head pair g = c%4
(heads 2g, 2g+1 -> D-slice [128g, 128g+128)). Each core computes
    partial_out = concat_h( softmax(causal(Q_h K_h^T / 8)) V_h ) @ W_O[slice]
for its two heads; the host sums the 4 partials per batch and adds b_O.

Pipeline design (v5). ScalarE exp() is the hard floor: 144 score-slot
ACTIVATEs totalling ~156us busy at (N+352)cyc/1.2GHz. Everything else
is organised so the exp stream starts early and never starves:

  - Score slot = ONE key block x BOTH heads in one [128,1024] PSUM tile
    (head A bank 0, head B bank 1), one exp ACTIVATE per slot through a
    [128,2,n] strided AP -> 144 calls.  Global unit stream: scores run
    1 unit ahead of exp, PV 1 unit behind, across slice boundaries.
  - zfinish(s): the zaug PSUM banks are released FAST by two bf16 CAST
    evacs [65,512] (L travels in bf16, ~0.4% scale err, fine vs the 2%
    gate); normalisation happens off the critical path: DVE reciprocal
    of the L row, an SBUF->SBUF 0-stride-partition DMA broadcasts 1/L
    across 64 partitions, and fused tensor_mul builds a pre-normalised
    [128,512] bf16 zpair (head B partition-shifted by a gpsimd DMA).
    The O-projection is then a SINGLE 128-contraction matmul per
    q-tile (both heads summed by the PE) plus one CAST evac.
  - Tail (slice 7): same chain but head B multiplies against a
    duplicated W_O[64:128] tile at partitions 0:64 (two serial
    accumulating matmuls) so the partition-shift DMA is off the
    critical path; evacs ride the then-idle ScalarE.
  - DMA plan: per-slice batched x^T descriptors ([128,4,512], one DMA
    each).  wq rides the Scalar-engine HWDGE queue (idle before the
    first ACTIVATE), ident/mask/biases/wk/x0-2/wv/wo on the sync queue,
    x3-7 + bvrep + wo2 + zpair stacking + output tiles on the GpSimd
    SWDGE queue.  Slice-0 projections are emitted before the stream
    (the PE is otherwise idle while DMAs land); projections for slice
    s+1 are spread one small item per stream iteration.
  - Masked scores use NEG=-640 (exp(-80)~1e-35): mathematically zero
    but safe for a future int16 fast-exp path.
"""

import numpy as np

import concourse.bass as bass
import concourse.mybir as mybir
from concourse.tile import TileContext
from concourse.bass_utils import run_bass_kernel_spmd

try:
    import ml_dtypes

    _BF16 = ml_dtypes.bfloat16
except ImportError:  # pragma: no cover
    _BF16 = None

F32 = mybir.dt.float32
BF16 = mybir.dt.bfloat16

B, T, D, H = 2, 4096, 512, 8
HD = D // H  # 64
SW = 512  # q-slice width
NS = T // SW  # 8 q-slices
NKC = D // 128  # 4 contraction chunks for the projections
NTT = T // 128  # 32 t-tiles / key blocks
NEG = -640.0  # masked score offset: exp((s+NEG)/8) <= exp(-74) ~ 0


def _split_waits(nc, max_waits=1):
    """The staged walrus rejects >1 semaphore wait per instruction; hoist
    extras onto same-engine NoOps inserted right before the instruction."""
    counter = 0
    for f in nc.m.functions:
        for blk in f.blocks:
            insts = blk.instructions
            out, changed = [], False
            for ins in insts:
                si = getattr(ins, "sync_info", None)
                waits = list(si.on_wait) if si is not None and si.on_wait else []
                if len(waits) > max_waits:
                    changed = True
                    for w in waits[:-max_waits]:
                        counter += 1
                        nop = mybir.InstNoOp(
                            name=f"I-wsplit-{counter}",
                            engine=ins.engine,
                            ins=[],
                            outs=[],
                        )
                        nop.sync_info = mybir.SyncInfo(on_wait=[w], on_update=[])
                        out.append(nop)
                    ins.sync_info = mybir.SyncInfo(
                        on_wait=waits[-max_waits:], on_update=list(si.on_update)
                    )
                out.append(ins)
            if changed:
                blk.instructions = out
    return counter


def _bcast_partitions(ap, n):
    """Replace the partition dim of an AP with an n-wide 0-stride dim."""
    return bass.AP(
        tensor=ap.tensor, offset=ap.offset, ap=[[0, n]] + list(ap.ap[1:])
    )


def build_nc():
    nc = bass.Bass("TRN2")

    xt = nc.dram_tensor("xt", [D, T], BF16, kind="ExternalInput")
    wq = nc.dram_tensor("wq", [D, 128], BF16, kind="ExternalInput")
    wk = nc.dram_tensor("wk", [D, 128], BF16, kind="ExternalInput")
    wv = nc.dram_tensor("wv", [D, 128], BF16, kind="ExternalInput")
    wo = nc.dram_tensor("wo", [128, D], BF16, kind="ExternalInput")
    bq = nc.dram_tensor("bq", [128, 1], F32, kind="ExternalInput")
    bk = nc.dram_tensor("bk", [128, 1], F32, kind="ExternalInput")
    bv = nc.dram_tensor("bv", [1, 128], F32, kind="ExternalInput")
    out = nc.dram_tensor("out", [T, D], BF16, kind="ExternalOutput")

    # maskneg[k, q'] = 0 where q' >= k else NEG  (S^T diagonal subtile mask)
    mask_np = np.where(
        np.arange(128)[None, :] >= np.arange(128)[:, None], 0.0, NEG
    ).astype(np.float32)
    ident_np = np.eye(128, dtype=np.float32)
    ident_dram = nc.inline_tensor(ident_np.astype(_BF16), name="identc")
    mask_dram = nc.inline_tensor(mask_np.astype(_BF16), name="maskc")

    with TileContext(nc) as tc:
        with (
            tc.tile_pool(name="singles", bufs=1) as singles,
            tc.tile_pool(name="ps", bufs=2, space="PSUM") as ps,
            tc.tile_pool(name="ex", bufs=2, space="PSUM") as ext,
            tc.tile_pool(name="zps", bufs=1, space="PSUM") as zps,
            tc.tile_pool(name="pt", bufs=6) as ptp,
            tc.tile_pool(name="rb", bufs=2) as rbp,
            tc.tile_pool(name="rr", bufs=4) as rrp,
            tc.tile_pool(name="zn", bufs=2) as znp,
            tc.tile_pool(name="outp", bufs=3) as outp,
            tc.tile_pool(name="drp", bufs=2, space="DRAM") as drp,
        ):
            # ---------- DMA schedule ----------
            # scalar HWDGE queue (idle until first ACTIVATE): wq only, so
            # the ACT_TABLE_LOAD that precedes the warm exp lands early.
            wq_sb = singles.tile([128, NKC, 128], BF16, tag="wq")
            nc.scalar.dma_start(
                out=wq_sb[:, :, :], in_=wq[:, :].rearrange("(c p) n -> p c n", p=128)
            )
            # sync queue: ident, mask, biases, wk, x0, wv, x1, x2, wo
            ident_sb = singles.tile([128, 128], BF16, tag="ident")
            mask_sb = singles.tile([128, 128], BF16, tag="mask")
            nc.sync.dma_start(out=ident_sb[:, :], in_=ident_dram[:, :])
            nc.sync.dma_start(out=mask_sb[:, :], in_=mask_dram[:, :])
            bq_sb = singles.tile([128, 1], F32, tag="bq")
            bk_sb = singles.tile([128, 1], F32, tag="bk")
            nc.sync.dma_start(out=bq_sb[:, :], in_=bq[:, :])
            nc.sync.dma_start(out=bk_sb[:, :], in_=bk[:, :])
            wk_sb = singles.tile([128, NKC, 128], BF16, tag="wk")
            nc.sync.dma_start(
                out=wk_sb[:, :, :], in_=wk[:, :].rearrange("(c p) n -> p c n", p=128)
            )
            xts = [
                singles.tile([128, NKC, SW], BF16, tag=f"xts{s}", name=f"xts{s}")
                for s in range(NS)
            ]
            nc.sync.dma_start(
                out=xts[0][:, :, :],
                in_=xt[:, 0:SW].rearrange("(c p) t -> p c t", p=128),
            )
            wv_sb = singles.tile([128, NKC, 128], BF16, tag="wv")
            nc.sync.dma_start(
                out=wv_sb[:, :, :], in_=wv[:, :].rearrange("(c p) n -> p c n", p=128)
            )
            for s in (1, 2):
                nc.sync.dma_start(
                    out=xts[s][:, :, :],
                    in_=xt[:, s * SW : (s + 1) * SW].rearrange(
                        "(c p) t -> p c t", p=128
                    ),
                )
            wo_sb = singles.tile([128, D], BF16, tag="wo")
            nc.sync.dma_start(out=wo_sb[:, :], in_=wo[:, :])
            # gpsimd SWDGE queue: x3, bvrep, x4..x7, wo2
            nc.gpsimd.dma_start(
                out=xts[3][:, :, :],
                in_=xt[:, 3 * SW : 4 * SW].rearrange("(c p) t -> p c t", p=128),
            )
            bvrep_sb = singles.tile([128, 128], F32, tag="bvrep")
            nc.gpsimd.dma_start(out=bvrep_sb[:, :], in_=_bcast_partitions(bv[:, :], 128))
            for s in range(4, NS):
                nc.gpsimd.dma_start(
                    out=xts[s][:, :, :],
                    in_=xt[:, s * SW : (s + 1) * SW].rearrange(
                        "(c p) t -> p c t", p=128
                    ),
                )
            # W_O rows 64:128 duplicated at partitions 0:64 (tail head-B rhs)
            wo2_sb = singles.tile([HD, D], BF16, tag="wo2")
            nc.gpsimd.dma_start(out=wo2_sb[:, :], in_=wo[HD:128, :])

            # preload the exp table set while DMAs stream in (the
            # ACT_TABLE_LOAD pseudo-inst lands before this in queue order)
            warm_sb = singles.tile([1, 1], BF16, tag="warm")
            nc.scalar.activation(
                out=warm_sb[:, :],
                in_=bq_sb[0:1, 0:1],
                func=mybir.ActivationFunctionType.Exp,
                scale=0.125,
            )

            qt_sb = [
                singles.tile([128, SW], BF16, tag=f"qt{s}", name=f"qt_sb{s}")
                for s in range(NS)
            ]
            kt_sb = [
                singles.tile([128, SW], BF16, tag=f"kt{s}", name=f"kt_sb{s}")
                for s in range(NS)
            ]
            # V_aug pair per key block: [128(t), 130]; cols 0:64 head A,
            # col 64 ones(A), cols 65:129 head B, col 129 ones(B)
            va_sb = [
                singles.tile([128, 2 * (HD + 1)], BF16, tag=f"va{t}", name=f"va_sb{t}")
                for t in range(NTT)
            ]
            for t in range(NTT):
                nc.vector.memset(va_sb[t][:, HD : HD + 1], 1.0)
                nc.vector.memset(va_sb[t][:, 2 * HD + 1 : 2 * HD + 2], 1.0)

            hrows = (slice(0, HD), slice(HD, 128))

            # ---------- emit helpers ----------
            def emit_proj_q(s, w_sb=None, b_sb=None, dst=None):
                w_sb = w_sb if w_sb is not None else wq_sb
                sg = ext.tile([128, SW], F32, tag="ex", name="ps_q")
                for c in range(NKC):
                    nc.tensor.matmul(
                        sg[:, :],
                        lhsT=w_sb[:, c, :],
                        rhs=xts[s][:, c, :],
                        start=(c == 0),
                        stop=(c == NKC - 1),
                        skip_group_check=True,
                    )
                nc.vector.tensor_scalar_add(
                    (dst if dst is not None else qt_sb[s])[:, :],
                    sg[:, :],
                    (b_sb if b_sb is not None else bq_sb)[:, :],
                )

            def emit_proj_k(s):
                emit_proj_q(s, w_sb=wk_sb, b_sb=bk_sb, dst=kt_sb[s])

            def emit_proj_v_tt(s, tt):
                # one ex tile (1 bank) per t-tile so the DVE evac of tile N
                # never reads a bank the PE is still writing (fatal)
                t = 4 * s + tt
                sg = ext.tile([128, SW], F32, tag="ex", name="ps_v")
                for c in range(NKC):
                    nc.tensor.matmul(
                        sg[:, 0:128],
                        lhsT=xts[s][:, c, tt * 128 : (tt + 1) * 128],
                        rhs=wv_sb[:, c, :],
                        start=(c == 0),
                        stop=(c == NKC - 1),
                        skip_group_check=True,
                    )
                # evac + b_V add in one op: dst [128,2,64] strided
                dst3 = va_sb[t][:, 0 : 2 * (HD + 1)].rearrange(
                    "p (a b) -> p a b", a=2
                )[:, :, 0:HD]
                src3 = sg[:, 0:128].rearrange("p (a b) -> p a b", a=2)
                bv3 = bvrep_sb[:, :].rearrange("p (a b) -> p a b", a=2)
                nc.vector.tensor_add(dst3, src3, bv3)

            def emit_scores(unit):
                s, kb, n, qlo = unit[:4]
                qs = s * SW
                diag = kb * 128 >= qs
                sg = ps.tile([128, 2 * SW], F32, tag="sg", name="ps_sg")
                unit[4] = sg
                for h in range(2):
                    off = h * SW
                    nc.tensor.matmul(
                        sg[:, off : off + n],
                        lhsT=kt_sb[kb // 4][
                            hrows[h], (kb % 4) * 128 : (kb % 4 + 1) * 128
                        ],
                        rhs=qt_sb[s][hrows[h], qlo - qs : qlo - qs + n],
                        start=True,
                        stop=not diag,
                        skip_group_check=True,
                        tile_position=(h * HD, 0),
                    )
                if diag:
                    for h in range(2):
                        nc.tensor.matmul(
                            sg[:, h * SW : h * SW + 128],
                            lhsT=ident_sb[:, :],
                            rhs=mask_sb[:, :],
                            start=False,
                            stop=True,
                            skip_group_check=True,
                        )

            def emit_exp(unit):
                s, kb, n, qlo, sg = unit[:5]
                pt = ptp.tile([128, 2 * SW], BF16, tag="pt", name="pt")
                in3 = sg[:, :].rearrange("p (a b) -> p a b", a=2)[:, :, 0:n]
                out3 = pt[:, 0 : 2 * n].rearrange("p (a b) -> p a b", a=2)
                nc.scalar.activation(
                    out=out3,
                    in_=in3,
                    func=mybir.ActivationFunctionType.Exp,
                    scale=0.125,
                )
                unit[4] = pt

            def emit_pv(unit, zaug):
                s, kb, n, qlo, pt = unit[:5]
                qs = s * SW
                nkb = 4 * (s + 1)
                for h in range(2):
                    nc.tensor.matmul(
                        zaug[h][0 : HD + 1, qlo - qs : SW],
                        lhsT=va_sb[kb][:, h * (HD + 1) : (h + 1) * (HD + 1)],
                        rhs=pt[:, h * n : h * n + n],
                        start=(kb == 0),
                        stop=(kb == nkb - 1),
                        skip_group_check=True,
                    )

            # ---------- zfinish: fast PSUM release + pre-normalised zpair ----
            # oproj work items: (zpair, qs, j, ready_at)
            oproj_work = []

            def emit_zfinish(sp, zaug, g0=0):
                # 1) bf16 CAST evacs release the zaug banks ASAP (the next
                #    slice's first PV WAR-waits on these reads)
                zraw = []
                for h in range(2):
                    zr = znp.tile(
                        [HD + 1, SW], BF16, tag=f"zraw{h}", name=f"zraw{h}"
                    )
                    nc.vector.tensor_copy(zr[:, :], zaug[h][:, :])
                    zraw.append(zr)
                # 2) off-critical-path: 1/L, partition-broadcast, normalise
                rbc = rbp.tile([128, SW], F32, tag="rb", name="rbc")
                for h in range(2):
                    rr = rrp.tile([1, SW], F32, tag="rr", name="rrow")
                    nc.vector.reciprocal(rr[0:1, :], zraw[h][HD : HD + 1, :])
                    rd = drp.tile([1, SW], F32, tag=f"rd{h}", name="rd")
                    nc.sync.dma_start(out=rd[:, :], in_=rr[0:1, :])
                    nc.sync.dma_start(
                        out=rbc[h * HD : (h + 1) * HD, :],
                        in_=_bcast_partitions(rd[:, :], HD),
                    )
                zpair = znp.tile([128, SW], BF16, tag="zp", name="zpair")
                zsh = znp.tile([128, SW], BF16, tag="zs", name="zsh")
                # head B partition-shift (gpsimd DMA), then aligned muls
                nc.gpsimd.dma_start(out=zsh[HD:128, :], in_=zraw[1][0:HD, :])
                nc.vector.tensor_mul(
                    zpair[0:HD, :], zraw[0][0:HD, :], rbc[0:HD, :]
                )
                nc.vector.tensor_mul(
                    zpair[HD:128, :], zsh[HD:128, :], rbc[HD:128, :]
                )
                for j in range(4):
                    oproj_work.append((zpair, sp * SW, j, g0 + 6 + 2 * j))

            def emit_oproj_qtile(item):
                zpair, qs_t, j, _ = item
                op = ext.tile([128, SW], F32, tag="ex", name="ps_o")
                jq = slice(j * 128, (j + 1) * 128)
                # both heads in ONE 128-contraction matmul (wo_sb rows are
                # the two heads' W_O rows stacked; zpair is pre-normalised)
                nc.tensor.matmul(
                    op[:, :],
                    lhsT=zpair[:, jq],
                    rhs=wo_sb[:, :],
                    start=True,
                    stop=True,
                    skip_group_check=True,
                )
                o_sb = outp.tile([128, D], BF16, tag="ot", name="o_sb")
                nc.vector.tensor_copy(o_sb[:, :], op[:, :])
                r0 = qs_t + j * 128
                nc.gpsimd.dma_start(out=out[r0 : r0 + 128, :], in_=o_sb[:, :])

            # HAM warm-up bridging preamble -> first projection
            warm_ps = ext.tile([128, SW], F32, tag="ex", name="ps_warm")
            for _ in range(20):
                nc.tensor.matmul(
                    warm_ps[:, 0:128],
                    lhsT=ident_sb[:, :],
                    rhs=mask_sb[:, :],
                    start=True,
                    stop=True,
                    skip_group_check=True,
                )

            # ---------- global unit stream ----------
            stream = []
            first_of_slice = {}
            for s in range(NS):
                qs = s * SW
                first_of_slice[s] = len(stream)
                for kb in range(4 * (s + 1)):
                    qlo = max(qs, kb * 128)
                    stream.append([s, kb, qs + SW - qlo, qlo, None])
            G = len(stream)

            def proj_items(sn):
                return [
                    lambda sn=sn: emit_proj_q(sn),
                    lambda sn=sn: emit_proj_k(sn),
                    lambda sn=sn: emit_proj_v_tt(sn, 0),
                    lambda sn=sn: emit_proj_v_tt(sn, 1),
                    lambda sn=sn: emit_proj_v_tt(sn, 2),
                    lambda sn=sn: emit_proj_v_tt(sn, 3),
                ]

            pending_proj = []  # for the upcoming slice

            def inserts(s, i, g, L):
                # 1) projection items for slice s+1 (hard deadline: all
                #    emitted before slice s+1 begins); 2/iter for short
                #    slices, else 1/iter starting at i=1
                budget = 2 if L <= 8 else 1
                did = 0
                while pending_proj and did < budget:
                    left = L - i
                    if len(pending_proj) >= left or i >= 1:
                        pending_proj.pop(0)()
                        did += 1
                    else:
                        break
                # 2) O-proj q-tiles when pacing allows and no proj emitted
                if did == 0 and oproj_work and g >= oproj_work[0][3]:
                    emit_oproj_qtile(oproj_work.pop(0))

            # prefill: slice-0 Q/K then the first scores (so exp starts
            # ASAP), then slice-0 V while exp(u0) runs
            emit_proj_q(0)
            emit_proj_k(0)
            emit_scores(stream[0])
            for tt in range(4):
                emit_proj_v_tt(0, tt)

            zaug = None
            prev_zaug = None
            for g in range(G):
                s, kb = stream[g][0], stream[g][1]
                L = 4 * (s + 1)
                i = g - first_of_slice[s]
                if i == 0 and s + 1 < NS:
                    pending_proj.extend(proj_items(s + 1))
                emit_exp(stream[g])
                if g + 1 < G:
                    emit_scores(stream[g + 1])
                if g >= 1:
                    ps_, pkb = stream[g - 1][0], stream[g - 1][1]
                    if pkb == 0:
                        # stream[g-1] opens slice ps_: finish the previous
                        # slice's accumulators, then claim fresh ones
                        if prev_zaug is not None:
                            emit_zfinish(ps_ - 1, prev_zaug, g0=g)
                        zaug = [
                            zps.tile([HD + 1, SW], F32, tag="za", name="zauga"),
                            zps.tile([HD + 1, SW], F32, tag="zb", name="zaugb"),
                        ]
                        prev_zaug = zaug
                    emit_pv(stream[g - 1], zaug)
                inserts(s, i, g, L)
            emit_pv(stream[G - 1], zaug)

            # ---------- tail: slice 7 ----------
            # Same zfinish chain, but head B goes through wo2 (W_O rows
            # 64:128 at partitions 0:64) -> no partition-shift DMA on the
            # critical path; evacs ride the now-idle ScalarE.
            zraw = []
            for h in range(2):
                zr = znp.tile([HD + 1, SW], BF16, tag=f"zraw{h}", name=f"zrawt{h}")
                nc.vector.tensor_copy(zr[:, :], zaug[h][:, :])
                zraw.append(zr)
            rbc2 = [
                rbp.tile([HD, SW], F32, tag=f"rb2{h}", name=f"rbc2{h}")
                for h in range(2)
            ]
            zn2 = []
            for h in range(2):
                rr = rrp.tile([1, SW], F32, tag="rr", name="rrowt")
                nc.vector.reciprocal(rr[0:1, :], zraw[h][HD : HD + 1, :])
                rd = drp.tile([1, SW], F32, tag=f"rd{h}", name="rdt")
                nc.sync.dma_start(out=rd[:, :], in_=rr[0:1, :])
                nc.sync.dma_start(
                    out=rbc2[h][0:HD, :], in_=_bcast_partitions(rd[:, :], HD)
                )
                zn = znp.tile([HD, SW], BF16, tag=f"zn2{h}", name=f"zn2{h}")
                nc.vector.tensor_mul(zn[0:HD, :], zraw[h][0:HD, :], rbc2[h][0:HD, :])
                zn2.append(zn)
            for j in range(4):
                jq = slice(j * 128, (j + 1) * 128)
                op = ext.tile([128, SW], F32, tag="ex", name="ps_ot")
                nc.tensor.matmul(
                    op[:, :],
                    lhsT=zn2[0][0:HD, jq],
                    rhs=wo_sb[0:HD, :],
                    start=True,
                    stop=False,
                    skip_group_check=True,
                )
                nc.tensor.matmul(
                    op[:, :],
                    lhsT=zn2[1][0:HD, jq],
                    rhs=wo2_sb[0:HD, :],
                    start=False,
                    stop=True,
                    skip_group_check=True,
                )
                o_sb = outp.tile([128, D], BF16, tag="ot", name="o_sbt")
                nc.scalar.copy(o_sb[:, :], op[:, :])
                r0 = (NS - 1) * SW + j * 128
                nc.gpsimd.dma_start(out=out[r0 : r0 + 128, :], in_=o_sb[:, :])
            # drain any remaining O-proj backlog (slice 6)
            while oproj_work:
                emit_oproj_qtile(oproj_work.pop(0))

    _split_waits(nc)
    return nc


_NC_CACHE = {}


def _get_nc():
    if "nc" not in _NC_CACHE:
        _NC_CACHE["nc"] = build_nc()
    return _NC_CACHE["nc"]


def make_in_maps(combined_embed, W_K, b_K, W_Q, b_Q, W_V, b_V, W_O, b_O):
    f32 = np.float32
    in_maps = []
    for c in range(8):
        b = c // 4
        g = c % 4
        sl = slice(g * 128, (g + 1) * 128)
        xt = np.ascontiguousarray(np.asarray(combined_embed[b], f32).T)
        in_maps.append(
            {
                "xt": xt.astype(_BF16),
                "wq": np.ascontiguousarray(np.asarray(W_Q, f32)[:, sl]).astype(_BF16),
                "wk": np.ascontiguousarray(np.asarray(W_K, f32)[:, sl]).astype(_BF16),
                "wv": np.ascontiguousarray(np.asarray(W_V, f32)[:, sl]).astype(_BF16),
                "wo": np.ascontiguousarray(np.asarray(W_O, f32)[sl, :]).astype(_BF16),
                "bq": np.asarray(b_Q, f32)[sl].reshape(128, 1).copy(),
                "bk": np.asarray(b_K, f32)[sl].reshape(128, 1).copy(),
                "bv": np.asarray(b_V, f32)[sl].reshape(1, 128).copy(),
            }
        )
    return in_maps


def run_cores(in_maps, **kwargs):
    nc = _get_nc()
    return run_bass_kernel_spmd(nc, in_maps, core_ids=list(range(8)), **kwargs)


def kernel(
    combined_embed, W_K, b_K, W_Q, b_Q, W_V, b_V, W_O, b_O
):  # full inputs -> full output
    in_maps = make_in_maps(
        combined_embed, W_K, b_K, W_Q, b_Q, W_V, b_V, W_O, b_O
    )
    res = run_cores(in_maps)
    out = np.zeros((B, T, D), np.float32)
    for c in range(8):
        out[c // 4] += np.asarray(res.results[c]["out"], np.float32)
    out += np.asarray(b_O, np.float32)[None, None, :]
    return out
